# revision 18
# baseline (speedup 1.0000x reference)
"""Multi-head causal attention (b=2, T=2048, d=1024, 16 heads) on 8 TRN2 cores.

Sharding: tensor-parallel over heads, 2 heads per core, both batch elements on
every core.  Per core:
  - QKV projections for its 2 heads (contraction over d_in=1024), with x^T
    resident in SBUF so Q^T/K^T come out in [channel, token] layout; V in
    natural [token, channel] layout augmented with a ones column.
  - Scores in transposed layout S^T[kpos, q]; diagonal tiles shrink N to the
    causally-valid q suffix, and only the leading 128-column chunk of each
    diagonal tile needs the triangular mask.
  - attn @ V computed transposed: ctx[q, d] accumulated per 128-token chunk
    (matmul N = head_dim + 1), so the softmax denominator sits in a PSUM
    column and normalization is a per-partition tensor_scalar multiply.
  - Two 8-core AllToAlls (one per head) re-shard ctx[q, d] windows from
    head-sharded to token-sharded; landed tiles are transposed back to
    [d, q] on the tensor engine (identity transpose) for the out projection.
  - out = ctx @ Wo + bo per 512-token window; the h0-channel half of the
    projection (K=64 accumulation) overlaps AllToAll #2.
Host side only shards/casts inputs and concatenates the 8 output windows.
"""

import sys

sys.path.insert(0, "/opt/trn_rl_repo")

import numpy as np
import ml_dtypes

import concourse.bass as bass
import concourse.mybir as mybir
import concourse.tile as tile
from concourse.tile import add_dep_helper
from concourse import bacc
from concourse.bass_utils import run_bass_kernel_spmd

B = 2
T = 2048
D = 1024
DH = 64
HL = 2  # heads per core
P = 128
CI = D // P  # 8 contraction subtiles
TQ = B * T  # 4096
QB = 512  # q block
NQB = T // QB  # 4 q blocks per batch
NKT = T // P  # 16 kpos tiles per batch
NW = 8  # output windows == cores
NQT = QB // P  # 4 q 128-chunks per window
F32 = mybir.dt.float32
BF16 = mybir.dt.bfloat16
EXP = mybir.ActivationFunctionType.Exp

_CACHE = {}


def _build():
    nc = bacc.Bacc("TRN2", target_bir_lowering=False, num_devices=8)
    xt = nc.dram_tensor("xt", [D, TQ], BF16, kind="ExternalInput")
    wq = nc.dram_tensor("wq", [P, CI, P], BF16, kind="ExternalInput")
    wk = nc.dram_tensor("wk", [P, CI, P], BF16, kind="ExternalInput")
    wv = nc.dram_tensor("wv", [P, CI, P], BF16, kind="ExternalInput")
    wo = nc.dram_tensor("wo", [P, CI, D], BF16, kind="ExternalInput")
    bob = nc.dram_tensor("bob", [P, D], F32, kind="ExternalInput")
    msk = nc.dram_tensor("msk", [P, P], BF16, kind="ExternalInput")
    idn = nc.dram_tensor("idn", [P, P], BF16, kind="ExternalInput")
    out = nc.dram_tensor("out", [QB, D], F32, kind="ExternalOutput")

    xt_r = xt.rearrange("(s p) t -> p s t", p=P)

    with tile.TileContext(nc) as tc:
        with (
            tc.tile_pool(name="const", bufs=1) as const,
            tc.tile_pool(name="dram", bufs=1, space="DRAM") as dram,
        ):
            xt_sb = const.tile([P, CI, TQ], BF16)
            wq_sb = const.tile([P, CI, P], BF16)
            wk_sb = const.tile([P, CI, P], BF16)
            wv_sb = const.tile([P, CI, P], BF16)
            wo_sb = const.tile([P, CI, D], BF16)
            bob_sb = const.tile([P, D], F32)
            msk_sb = const.tile([P, P], BF16)
            idn_sb = const.tile([P, P], BF16)
            q_sb = const.tile([P, TQ], BF16)
            k_sb = const.tile([P, TQ], BF16)
            # V augmented with a trailing ones column (softmax denominator)
            v_sb = const.tile([P, 2 * NKT, HL, DH + 1], BF16)
            # normalized ctx[q, d] per (head, window, q-chunk)
            stg = const.tile([P, HL, NW, NQT, DH], BF16)
            # landed ctx[q, d] tiles for my window: (head, source pair, chunk)
            cfq = const.tile([P, HL, NW, NQT, DH], BF16)
            # transposed ctx^T[d, q] for the out projection; h=0 channels on
            # partitions 0..63, h=1 on 64..127 (global channel 128j+64h+d)
            cf_sb = const.tile([P, CI, QB], BF16)

            # token-chunked x^T DMAs, chained so chunk t8 arrives at ~t8/8 of
            # the transfer; phase A consumes chunks in the same order
            nc.sync.dma_start(wq_sb[:], wq[:])
            nc.sync.dma_start(xt_sb[:, :, 0:P], xt_r[:, :, 0:P])
            nc.sync.dma_start(wk_sb[:], wk[:])
            nc.sync.dma_start(wv_sb[:], wv[:])
            nc.sync.dma_start(msk_sb[:], msk[:])
            xt_spans = [(sub * P, (sub + 1) * P) for sub in range(1, 4)] + [
                (t8 * QB, (t8 + 1) * QB) for t8 in range(1, TQ // QB)
            ]
            for lo, hi in xt_spans:
                nc.sync.dma_start(xt_sb[:, :, lo:hi], xt_r[:, :, lo:hi])
            nc.sync.dma_start(wo_sb[:], wo[:])
            nc.sync.dma_start(bob_sb[:], bob[:])
            nc.sync.dma_start(idn_sb[:], idn[:])
            nc.vector.memset(v_sb[:, :, :, DH : DH + 1], 1.0)

            # ---- Phases 1+2: QKV waves interleaved with attention ----
            # Phase 1: per 512-token wave w, project Q/K/V for that token
            # chunk (all channels), then run h0's attention window w.  This
            # spreads the Act-engine exp load over the QKV span.  Phase 2:
            # h1's attention windows.  One AllToAll per head as before.
            a2a1_in = dram.tile([NW, P, NQT, DH], BF16)
            a2a1_out = dram.tile([NW, P, NQT, DH], BF16)
            a2a2_in = dram.tile([NW, P, NQT, DH], BF16)
            a2a2_out = dram.tile([NW, P, NQT, DH], BF16)
            a2a = ((a2a1_in, a2a1_out), (a2a2_in, a2a2_out))
            land_dma = [None, None]

            # h1 windows whose scores/exp run early, during phase 1 (the
            # Act engine idles there); their at tiles persist in SBUF
            PRE_W = (0, 1, 2, 4, 5, 6)
            PRE_WAVE = {}
            atp = const.tile([P, 24, 2 * QB], BF16)
            pre_at = {}  # (w, group index) -> atp slot

            def win_groups(qb):
                kd = 4 * qb
                gs = [
                    [(k2 * QB, QB, 2 * g + k2) for k2 in range(2)]
                    for g in range(2 * qb)
                ]
                gs.append([(0, 512, kd), (512, 384, kd + 1)])
                gs.append([(0, 256, kd + 2), (256, 128, kd + 3)])
                return gs

            with (
                tc.tile_pool(name="attn", bufs=4) as apool,
                tc.tile_pool(name="psQK", bufs=1, space="PSUM") as psQK,
                tc.tile_pool(name="psS", bufs=3, space="PSUM") as psS,
                tc.tile_pool(name="psCV", bufs=1, space="PSUM") as psCV,
                tc.tile_pool(name="nrm", bufs=2) as nrm,
            ):

                def score_group(h, b, qb, chunks, at=None):
                    hp, tb, qs, kd = DH * h, b * T, b * T + qb * QB, 4 * qb
                    tot = chunks[-1][0] + chunks[-1][1]
                    diag = chunks[0][2] >= kd
                    sps = psS.tile([P, 2 * QB], F32, tag="s", name="sps")
                    if at is None:
                        at = apool.tile([P, 2 * QB], BF16, tag="at", name="at")
                    for off, wN, kt in chunks:
                        qoff = QB - wN  # q columns qoff..512
                        nc.tensor.matmul(
                            sps[:, off : off + wN],
                            k_sb[hp : hp + DH, tb + kt * P : tb + (kt + 1) * P],
                            q_sb[hp : hp + DH, qs + qoff : qs + QB],
                            start=True,
                            stop=True,
                        )
                    nc.scalar.activation(
                        at[:, 0:tot], sps[:, 0:tot], EXP, scale=0.125
                    )
                    if diag:
                        # triangular mask on the leading 128 columns of each
                        # diagonal tile
                        for off, wN, kt in chunks:
                            nc.vector.tensor_mul(
                                at[:, off : off + P],
                                at[:, off : off + P],
                                msk_sb[:],
                            )
                    return at

                def attnv_group(at, h, b, qb, chunks, ctxq):
                    kb, kd = b * NKT, 4 * qb
                    for off, wN, kt in chunks:
                        qoff = QB - wN
                        for qt in range(NQT):
                            if qt * P < qoff:
                                continue  # causally empty
                            nc.tensor.matmul(
                                ctxq[:, qt, :],
                                at[:, off + qt * P - qoff : off + qt * P - qoff + P],
                                v_sb[:, kb + kt, h, :],
                                start=False,
                                stop=(kt == kd + qt),
                                skip_group_check=True,
                            )

                for h in range(HL):
                    hp = DH * h
                    for w in range(NW):
                        b, qb = w // NQB, w % NQB
                        tb = b * T
                        kb = b * NKT
                        qs = tb + qb * QB
                        t8s = slice(w * QB, (w + 1) * QB)
                        kd = 4 * qb  # first diagonal kt
                        # combined ctx + V-projection scratch: one PSUM bank.
                        # ctx chunks at [0:4,0:65]; V scratch at flat [260:388].
                        # start=True would zero the whole 2KB bank, so both
                        # regions are memset once and accumulate start=False.
                        cv = psCV.tile([P, 6, DH + 1], F32, tag="cv", name="cv")
                        cvf = cv[:].rearrange("p a b -> p (a b)")
                        ctxq = cv[:, 0:NQT, :]
                        nc.vector.memset(cvf[:, 0 : NQT * (DH + 1)], 0.0)

                        if h == 0:
                            # QKV projections for this token wave.  Wave 0
                            # runs in 128-column sub-groups so the first
                            # matmuls only wait on a 128-token slice of x^T.
                            for dst, wt, kk in ((q_sb, wq_sb, 0), (k_sb, wk_sb, 1)):
                                pt = psQK.tile([P, QB], F32, tag="qk", name="pt")
                                if w == 0:
                                    nc.vector.memset(pt[:], 0.0)
                                    for sub in range(4):
                                        ss = slice(sub * P, (sub + 1) * P)
                                        for sI in range(CI):
                                            nc.tensor.matmul(
                                                pt[:, ss],
                                                wt[:, sI, :],
                                                xt_sb[:, sI, ss],
                                                start=False,
                                                stop=(sI == CI - 1),
                                                skip_group_check=True,
                                            )
                                else:
                                    for sI in range(CI):
                                        nc.tensor.matmul(
                                            pt[:],
                                            wt[:, sI, :],
                                            xt_sb[:, sI, t8s],
                                            start=(sI == 0),
                                            stop=(sI == CI - 1),
                                        )
                                nc.vector.tensor_copy(dst[:, t8s], pt[:])
                            for tt4 in range(QB // P):
                                tt = w * (QB // P) + tt4
                                pvr = cvf[:, NQT * (DH + 1) : NQT * (DH + 1) + P]
                                nc.vector.memset(pvr, 0.0)
                                for sI in range(CI):
                                    nc.tensor.matmul(
                                        pvr,
                                        xt_sb[:, sI, tt * P : (tt + 1) * P],
                                        wv_sb[:, sI, :],
                                        start=False,
                                        stop=(sI == CI - 1),
                                        skip_group_check=True,
                                    )
                                nc.vector.tensor_copy(
                                    v_sb[:, tt, :, 0:DH],
                                    pvr.rearrange("p (h d) -> p h d", h=HL),
                                )

                        # attention window (h, b, qb)
                        for gi, chunks in enumerate(win_groups(qb)):
                            if h == 1 and (w, gi) in pre_at:
                                at = atp[:, pre_at[(w, gi)], :]
                            else:
                                at = score_group(h, b, qb, chunks)
                            attnv_group(at, h, b, qb, chunks, ctxq)
                        # normalize: per-partition reciprocal of the
                        # denominator column, then scale the d columns
                        rc = nrm.tile([P, NQT, 1], F32, tag="rc", name="rc")
                        nc.vector.reciprocal(rc[:], ctxq[:, :, DH : DH + 1])
                        for qt in range(NQT):
                            nc.vector.tensor_scalar_mul(
                                stg[:, h, w, qt, :],
                                ctxq[:, qt, 0:DH],
                                rc[:, qt, :],
                            )
                        # stage this window for the AllToAll right away
                        nc.sync.dma_start(a2a[h][0][w], stg[:, h, w, :, :])
                        if h == 0:
                            for pw, at_wave in PRE_WAVE.items():
                                if at_wave != w:
                                    continue
                                pb, pqb = pw // NQB, pw % NQB
                                for gi, chunks in enumerate(win_groups(pqb)):
                                    slot = len(pre_at)
                                    pre_at[(pw, gi)] = slot
                                    score_group(1, pb, pqb, chunks, at=atp[:, slot, :])
                    # all 8 windows staged; run this head's AllToAll
                    a_in, a_out = a2a[h]
                    nc.gpsimd.collective_compute(
                        "AllToAll",
                        mybir.AluOpType.bypass,
                        replica_groups=[[0, 1, 2, 3, 4, 5, 6, 7]],
                        ins=[a_in.opt()],
                        outs=[a_out.opt()],
                    )
                    a_out_r = a_out.rearrange("j p a c -> p j a c")
                    nc.sync.dma_start(cfq[:, h, 0:4, :, :], a_out_r[:, 0:4])
                    land_dma[h] = nc.sync.dma_start(
                        cfq[:, h, 4:NW, :, :], a_out_r[:, 4:NW]
                    )
                    if h == 0:
                        # phase 1.5: h1 scores/exp for the precompute windows,
                        # running in A2A#1's shadow
                        for pw in PRE_W:
                            if pw in PRE_WAVE:
                                continue
                            pb, pqb = pw // NQB, pw % NQB
                            for gi, chunks in enumerate(win_groups(pqb)):
                                slot = len(pre_at)
                                pre_at[(pw, gi)] = slot
                                score_group(1, pb, pqb, chunks, at=atp[:, slot, :])

            # ---- Phase D: output projection for my token window ----
            # h0 transposes + D1 (h0 halves, K=64, all 8 units, evicted to
            # SBUF bf16 with the bias pre-added) overlap A2A#2; h1 transposes
            # + D2 (h1 halves) after it lands; final add + store per token
            # tile.  Transposes batch 4 q-chunks per source into one PSUM
            # bank (memset + start=False) so each source needs one copy.
            units = [(tt, n2) for tt in range(NQT) for n2 in range(2)]
            ev = const.tile([P, NQT, 2, 512], BF16)  # h0-half partials + bias
            with (
                tc.tile_pool(name="psO", bufs=6, space="PSUM") as psO,
                tc.tile_pool(name="osb", bufs=2) as osb,
                tc.tile_pool(name="psTT", bufs=2, space="PSUM") as psTT,
            ):
                for hh in range(2):
                    if hh == 1:
                        # D1 between the two transpose rounds (below)
                        for tt, n2 in units:
                            po = psO.tile([P, 512], F32, tag="po", name="po")
                            for sI in range(CI):
                                nc.tensor.matmul(
                                    po[:],
                                    cf_sb[0:DH, sI, tt * P : (tt + 1) * P],
                                    wo_sb[0:DH, sI, n2 * 512 : (n2 + 1) * 512],
                                    start=(sI == 0),
                                    stop=(sI == CI - 1),
                                )
                            nc.vector.tensor_add(
                                ev[:, tt, n2, :],
                                po[:],
                                bob_sb[:, n2 * 512 : (n2 + 1) * 512],
                            )
                    for j in range(NW):
                        for qt in range(NQT):
                            tr = psTT.tile([DH, P], BF16, tag="tr", name="tr")
                            nc.tensor.matmul(
                                tr[:],
                                cfq[:, hh, j, qt, :],
                                idn_sb[:],
                                is_transpose=True,
                            )
                            dst = cf_sb[hh * DH : hh * DH + DH, j, qt * P : (qt + 1) * P]
                            nc.scalar.copy(dst, tr[:])
                # D2: h1 halves; add the evicted h0 partials and store
                for tt in range(NQT):
                    ot = osb.tile([P, D], F32, tag="o", name="ot")
                    for n2 in range(2):
                        po = psO.tile([P, 512], F32, tag="po", name="po")
                        for sI in range(CI):
                            nc.tensor.matmul(
                                po[:],
                                cf_sb[DH:P, sI, tt * P : (tt + 1) * P],
                                wo_sb[DH:P, sI, n2 * 512 : (n2 + 1) * 512],
                                start=(sI == 0),
                                stop=(sI == CI - 1),
                            )
                        nc.vector.tensor_tensor(
                            ot[:, n2 * 512 : (n2 + 1) * 512],
                            po[:],
                            ev[:, tt, n2, :],
                            mybir.AluOpType.add,
                        )
                    nc.sync.dma_start(out[tt * P : (tt + 1) * P, :], ot[:])
    nc.finalize()
    return nc


def _get_nc():
    if "nc" not in _CACHE:
        _CACHE["nc"] = _build()
    return _CACHE["nc"]


def kernel(x, Wq, Wk, Wv, Wo, bo, **run_kwargs):
    x = np.asarray(x, np.float32)
    Wq = np.asarray(Wq, np.float32)
    Wk = np.asarray(Wk, np.float32)
    Wv = np.asarray(Wv, np.float32)
    Wo = np.asarray(Wo, np.float32)
    bo = np.asarray(bo, np.float32)

    xt16 = np.ascontiguousarray(x.reshape(TQ, D).T).astype(ml_dtypes.bfloat16)
    wo16 = np.ascontiguousarray(
        Wo.reshape(CI, P, D).transpose(1, 0, 2)
    ).astype(ml_dtypes.bfloat16)
    bob = np.ascontiguousarray(np.broadcast_to(bo, (P, D))).astype(np.float32)
    ii = np.arange(P)[:, None]
    jj = np.arange(P)[None, :]
    msk = (jj >= ii).astype(ml_dtypes.bfloat16)
    idn = np.eye(P).astype(ml_dtypes.bfloat16)

    in_maps = []
    for c in range(8):
        sl = slice(P * c, P * (c + 1))
        in_maps.append(
            {
                "xt": xt16,
                "wq": np.ascontiguousarray(
                    Wq[:, sl].reshape(CI, P, P).transpose(1, 0, 2)
                ).astype(ml_dtypes.bfloat16),
                "wk": np.ascontiguousarray(
                    Wk[:, sl].reshape(CI, P, P).transpose(1, 0, 2)
                ).astype(ml_dtypes.bfloat16),
                "wv": np.ascontiguousarray(
                    Wv[:, sl].reshape(CI, P, P).transpose(1, 0, 2)
                ).astype(ml_dtypes.bfloat16),
                "wo": wo16,
                "bob": bob,
                "msk": msk,
                "idn": idn,
            }
        )

    nc = _get_nc()
    res = run_bass_kernel_spmd(nc, in_maps, core_ids=list(range(8)), **run_kwargs)

    outp = np.empty((B, T, D), np.float32)
    for c in range(8):
        b, w = c // 4, c % 4
        outp[b, w * QB : (w + 1) * QB, :] = res.results[c]["out"]
    return outp


# revision 19
# speedup vs baseline: 1.1295x; 1.1295x over previous
"""Multi-head causal attention (b=2, T=2048, d=1024, 16 heads) on 8 TRN2 cores.

Sharding: tensor-parallel over heads, 2 heads per core, both batch elements on
every core.  Per core:
  - QKV projections for its 2 heads (contraction over d_in=1024), with x^T
    resident in SBUF so Q^T/K^T come out in [channel, token] layout; V in
    natural [token, channel] layout augmented with a ones column.
  - Scores in transposed layout S^T[kpos, q]; diagonal tiles shrink N to the
    causally-valid q suffix, and only the leading 128-column chunk of each
    diagonal tile needs the triangular mask.
  - attn @ V computed transposed: ctx[q, d] accumulated per 128-token chunk
    (matmul N = head_dim + 1), so the softmax denominator sits in a PSUM
    column and normalization is a per-partition tensor_scalar multiply.
  - Two 8-core AllToAlls (one per head) re-shard ctx[q, d] windows from
    head-sharded to token-sharded; landed tiles are transposed back to
    [d, q] on the tensor engine (identity transpose) for the out projection.
  - out = ctx @ Wo + bo per 512-token window; the h0-channel half of the
    projection (K=64 accumulation) overlaps AllToAll #2.
Host side only shards/casts inputs and concatenates the 8 output windows.
"""

import sys

sys.path.insert(0, "/opt/trn_rl_repo")

import numpy as np
import ml_dtypes

import concourse.bass as bass
import concourse.mybir as mybir
import concourse.tile as tile
from concourse.tile import add_dep_helper
from concourse import bacc
from concourse.bass_utils import run_bass_kernel_spmd

B = 2
T = 2048
D = 1024
DH = 64
HL = 2  # heads per core
P = 128
CI = D // P  # 8 contraction subtiles
TQ = B * T  # 4096
QB = 512  # q block
NQB = T // QB  # 4 q blocks per batch
NKT = T // P  # 16 kpos tiles per batch
NW = 8  # output windows == cores
NQT = QB // P  # 4 q 128-chunks per window
F32 = mybir.dt.float32
BF16 = mybir.dt.bfloat16
EXP = mybir.ActivationFunctionType.Exp

_CACHE = {}


def _build():
    nc = bacc.Bacc("TRN2", target_bir_lowering=False, num_devices=8)
    xt = nc.dram_tensor("xt", [D, TQ], BF16, kind="ExternalInput")
    wq = nc.dram_tensor("wq", [P, CI, P], BF16, kind="ExternalInput")
    wk = nc.dram_tensor("wk", [P, CI, P], BF16, kind="ExternalInput")
    wv = nc.dram_tensor("wv", [P, CI, P], BF16, kind="ExternalInput")
    wo = nc.dram_tensor("wo", [P, CI, D], BF16, kind="ExternalInput")
    bob = nc.dram_tensor("bob", [P, D], F32, kind="ExternalInput")
    msk = nc.dram_tensor("msk", [P, P], BF16, kind="ExternalInput")
    idn = nc.dram_tensor("idn", [P, P], BF16, kind="ExternalInput")
    out = nc.dram_tensor("out", [QB, D], F32, kind="ExternalOutput")

    xt_r = xt.rearrange("(s p) t -> p s t", p=P)

    with tile.TileContext(nc) as tc:
        with (
            tc.tile_pool(name="const", bufs=1) as const,
            tc.tile_pool(name="dram", bufs=1, space="DRAM") as dram,
        ):
            xt_sb = const.tile([P, CI, TQ], BF16)
            wq_sb = const.tile([P, CI, P], BF16)
            wk_sb = const.tile([P, CI, P], BF16)
            wv_sb = const.tile([P, CI, P], BF16)
            wo_sb = const.tile([P, CI, D], BF16)
            bob_sb = const.tile([P, D], F32)
            msk_sb = const.tile([P, P], BF16)
            idn_sb = const.tile([P, P], BF16)
            q_sb = const.tile([P, TQ], BF16)
            k_sb = const.tile([P, TQ], BF16)
            # V augmented with a trailing ones column (softmax denominator)
            v_sb = const.tile([P, 2 * NKT, HL, DH + 1], BF16)
            # normalized ctx[q, d] per (head, window, q-chunk)
            stg = const.tile([P, HL, NW, NQT, DH], BF16)
            # landed ctx[q, d] tiles for my window: (head, source pair, chunk)
            cfq = const.tile([P, HL, NW, NQT, DH], BF16)
            # transposed ctx^T[d, q] for the out projection; h=0 channels on
            # partitions 0..63, h=1 on 64..127 (global channel 128j+64h+d)
            cf_sb = const.tile([P, CI, QB], BF16)

            # token-chunked x^T DMAs, chained so chunk t8 arrives at ~t8/8 of
            # the transfer; phase A consumes chunks in the same order
            nc.sync.dma_start(wq_sb[:], wq[:])
            nc.sync.dma_start(xt_sb[:, :, 0:P], xt_r[:, :, 0:P])
            nc.sync.dma_start(wk_sb[:], wk[:])
            nc.sync.dma_start(wv_sb[:], wv[:])
            nc.sync.dma_start(msk_sb[:], msk[:])
            xt_spans = [(sub * P, (sub + 1) * P) for sub in range(1, 4)] + [
                (t8 * QB, (t8 + 1) * QB) for t8 in range(1, TQ // QB)
            ]
            for lo, hi in xt_spans:
                nc.sync.dma_start(xt_sb[:, :, lo:hi], xt_r[:, :, lo:hi])
            nc.sync.dma_start(wo_sb[:], wo[:])
            nc.sync.dma_start(bob_sb[:], bob[:])
            nc.sync.dma_start(idn_sb[:], idn[:])
            nc.vector.memset(v_sb[:, :, :, DH : DH + 1], 1.0)

            # ---- Phases 1+2: QKV waves interleaved with attention ----
            # Phase 1: per 512-token wave w, project Q/K/V for that token
            # chunk (all channels), then run h0's attention window w.  This
            # spreads the Act-engine exp load over the QKV span.  Phase 2:
            # h1's attention windows.  One AllToAll per head as before.
            a2a1_in = dram.tile([NW, P, NQT, DH], BF16)
            a2a1_out = dram.tile([NW, P, NQT, DH], BF16)
            a2a2_in = dram.tile([NW, P, NQT, DH], BF16)
            a2a2_out = dram.tile([NW, P, NQT, DH], BF16)
            a2a = ((a2a1_in, a2a1_out), (a2a2_in, a2a2_out))
            land_dma = [None, None]

            # h1 windows whose scores/exp run early, during phase 1 (the
            # Act engine idles there); their at tiles persist in SBUF
            PRE_W = (0, 1, 2, 4, 5, 6)
            PRE_WAVE = {}
            atp = const.tile([P, 24, 2 * QB], BF16)
            pre_at = {}  # (w, group index) -> atp slot

            def win_groups(qb):
                kd = 4 * qb
                gs = [
                    [(k2 * QB, QB, 2 * g + k2) for k2 in range(2)]
                    for g in range(2 * qb)
                ]
                gs.append([(0, 512, kd), (512, 384, kd + 1)])
                gs.append([(0, 256, kd + 2), (256, 128, kd + 3)])
                return gs

            with (
                tc.tile_pool(name="attn", bufs=4) as apool,
                tc.tile_pool(name="psQK", bufs=2, space="PSUM") as psQK,
                tc.tile_pool(name="psS", bufs=2, space="PSUM") as psS,
                tc.tile_pool(name="psCV", bufs=2, space="PSUM") as psCV,
                tc.tile_pool(name="nrm", bufs=2) as nrm,
            ):

                def score_group(h, b, qb, chunks, at=None):
                    hp, tb, qs, kd = DH * h, b * T, b * T + qb * QB, 4 * qb
                    tot = chunks[-1][0] + chunks[-1][1]
                    diag = chunks[0][2] >= kd
                    sps = psS.tile([P, 2 * QB], F32, tag="s", name="sps")
                    if at is None:
                        at = apool.tile([P, 2 * QB], BF16, tag="at", name="at")
                    for off, wN, kt in chunks:
                        qoff = QB - wN  # q columns qoff..512
                        nc.tensor.matmul(
                            sps[:, off : off + wN],
                            k_sb[hp : hp + DH, tb + kt * P : tb + (kt + 1) * P],
                            q_sb[hp : hp + DH, qs + qoff : qs + QB],
                            start=True,
                            stop=True,
                        )
                    nc.scalar.activation(
                        at[:, 0:tot], sps[:, 0:tot], EXP, scale=0.125
                    )
                    if diag:
                        # triangular mask on the leading 128 columns of each
                        # diagonal tile
                        for off, wN, kt in chunks:
                            nc.vector.tensor_mul(
                                at[:, off : off + P],
                                at[:, off : off + P],
                                msk_sb[:],
                            )
                    return at

                def attnv_group(at, h, b, qb, chunks, ctxq):
                    kb, kd = b * NKT, 4 * qb
                    for off, wN, kt in chunks:
                        qoff = QB - wN
                        for qt in range(NQT):
                            if qt * P < qoff:
                                continue  # causally empty
                            nc.tensor.matmul(
                                ctxq[:, qt, :],
                                at[:, off + qt * P - qoff : off + qt * P - qoff + P],
                                v_sb[:, kb + kt, h, :],
                                start=False,
                                stop=(kt == kd + qt),
                                skip_group_check=True,
                            )

                for h in range(HL):
                    hp = DH * h
                    for w in range(NW):
                        b, qb = w // NQB, w % NQB
                        tb = b * T
                        kb = b * NKT
                        qs = tb + qb * QB
                        t8s = slice(w * QB, (w + 1) * QB)
                        kd = 4 * qb  # first diagonal kt
                        # combined ctx + V-projection scratch: one PSUM bank.
                        # ctx chunks at [0:4,0:65]; V scratch at flat [260:388].
                        # start=True would zero the whole 2KB bank, so both
                        # regions are memset once and accumulate start=False.
                        cv = psCV.tile([P, 6, DH + 1], F32, tag="cv", name="cv")
                        cvf = cv[:].rearrange("p a b -> p (a b)")
                        ctxq = cv[:, 0:NQT, :]
                        nc.vector.memset(cvf[:, 0 : NQT * (DH + 1)], 0.0)

                        if h == 0:
                            # QKV projections for this token wave.  Wave 0
                            # runs in 128-column sub-groups so the first
                            # matmuls only wait on a 128-token slice of x^T.
                            for dst, wt, kk in ((q_sb, wq_sb, 0), (k_sb, wk_sb, 1)):
                                pt = psQK.tile([P, QB], F32, tag="qk", name="pt")
                                if w == 0:
                                    nc.vector.memset(pt[:], 0.0)
                                    for sub in range(4):
                                        ss = slice(sub * P, (sub + 1) * P)
                                        for sI in range(CI):
                                            nc.tensor.matmul(
                                                pt[:, ss],
                                                wt[:, sI, :],
                                                xt_sb[:, sI, ss],
                                                start=False,
                                                stop=(sI == CI - 1),
                                                skip_group_check=True,
                                            )
                                else:
                                    for sI in range(CI):
                                        nc.tensor.matmul(
                                            pt[:],
                                            wt[:, sI, :],
                                            xt_sb[:, sI, t8s],
                                            start=(sI == 0),
                                            stop=(sI == CI - 1),
                                        )
                                nc.vector.tensor_copy(dst[:, t8s], pt[:])
                            for tt4 in range(QB // P):
                                tt = w * (QB // P) + tt4
                                pvr = cvf[:, NQT * (DH + 1) : NQT * (DH + 1) + P]
                                nc.vector.memset(pvr, 0.0)
                                for sI in range(CI):
                                    nc.tensor.matmul(
                                        pvr,
                                        xt_sb[:, sI, tt * P : (tt + 1) * P],
                                        wv_sb[:, sI, :],
                                        start=False,
                                        stop=(sI == CI - 1),
                                        skip_group_check=True,
                                    )
                                nc.vector.tensor_copy(
                                    v_sb[:, tt, :, 0:DH],
                                    pvr.rearrange("p (h d) -> p h d", h=HL),
                                )

                        # attention window (h, b, qb)
                        for gi, chunks in enumerate(win_groups(qb)):
                            if h == 1 and (w, gi) in pre_at:
                                at = atp[:, pre_at[(w, gi)], :]
                            else:
                                at = score_group(h, b, qb, chunks)
                            attnv_group(at, h, b, qb, chunks, ctxq)
                        # normalize: per-partition reciprocal of the
                        # denominator column, then scale the d columns
                        rc = nrm.tile([P, NQT, 1], F32, tag="rc", name="rc")
                        nc.vector.reciprocal(rc[:], ctxq[:, :, DH : DH + 1])
                        for qt in range(NQT):
                            nc.vector.tensor_scalar_mul(
                                stg[:, h, w, qt, :],
                                ctxq[:, qt, 0:DH],
                                rc[:, qt, :],
                            )
                        # stage this window for the AllToAll right away
                        nc.sync.dma_start(a2a[h][0][w], stg[:, h, w, :, :])
                        if h == 0:
                            for pw, at_wave in PRE_WAVE.items():
                                if at_wave != w:
                                    continue
                                pb, pqb = pw // NQB, pw % NQB
                                for gi, chunks in enumerate(win_groups(pqb)):
                                    slot = len(pre_at)
                                    pre_at[(pw, gi)] = slot
                                    score_group(1, pb, pqb, chunks, at=atp[:, slot, :])
                    # all 8 windows staged; run this head's AllToAll
                    a_in, a_out = a2a[h]
                    nc.gpsimd.collective_compute(
                        "AllToAll",
                        mybir.AluOpType.bypass,
                        replica_groups=[[0, 1, 2, 3, 4, 5, 6, 7]],
                        ins=[a_in.opt()],
                        outs=[a_out.opt()],
                    )
                    a_out_r = a_out.rearrange("j p a c -> p j a c")
                    nc.sync.dma_start(cfq[:, h, 0:4, :, :], a_out_r[:, 0:4])
                    land_dma[h] = nc.sync.dma_start(
                        cfq[:, h, 4:NW, :, :], a_out_r[:, 4:NW]
                    )
                    if h == 0:
                        # phase 1.5: h1 scores/exp for the precompute windows,
                        # running in A2A#1's shadow
                        for pw in PRE_W:
                            if pw in PRE_WAVE:
                                continue
                            pb, pqb = pw // NQB, pw % NQB
                            for gi, chunks in enumerate(win_groups(pqb)):
                                slot = len(pre_at)
                                pre_at[(pw, gi)] = slot
                                score_group(1, pb, pqb, chunks, at=atp[:, slot, :])

            # ---- Phase D: output projection for my token window ----
            # h0 transposes + D1 (h0 halves, K=64, all 8 units, evicted to
            # SBUF bf16 with the bias pre-added) overlap A2A#2; h1 transposes
            # + D2 (h1 halves) after it lands; final add + store per token
            # tile.  Transposes batch 4 q-chunks per source into one PSUM
            # bank (memset + start=False) so each source needs one copy.
            units = [(tt, n2) for tt in range(NQT) for n2 in range(2)]
            ev = const.tile([P, NQT, 2, 512], BF16)  # h0-half partials + bias
            with (
                tc.tile_pool(name="psO", bufs=6, space="PSUM") as psO,
                tc.tile_pool(name="osb", bufs=2) as osb,
                tc.tile_pool(name="psTT", bufs=2, space="PSUM") as psTT,
            ):
                for hh in range(2):
                    if hh == 1:
                        # D1 between the two transpose rounds (below)
                        for tt, n2 in units:
                            po = psO.tile([P, 512], F32, tag="po", name="po")
                            for sI in range(CI):
                                nc.tensor.matmul(
                                    po[:],
                                    cf_sb[0:DH, sI, tt * P : (tt + 1) * P],
                                    wo_sb[0:DH, sI, n2 * 512 : (n2 + 1) * 512],
                                    start=(sI == 0),
                                    stop=(sI == CI - 1),
                                )
                            nc.vector.tensor_add(
                                ev[:, tt, n2, :],
                                po[:],
                                bob_sb[:, n2 * 512 : (n2 + 1) * 512],
                            )
                    for j in range(NW):
                        for qt in range(NQT):
                            tr = psTT.tile([DH, P], BF16, tag="tr", name="tr")
                            nc.tensor.matmul(
                                tr[:],
                                cfq[:, hh, j, qt, :],
                                idn_sb[:],
                                is_transpose=True,
                            )
                            dst = cf_sb[hh * DH : hh * DH + DH, j, qt * P : (qt + 1) * P]
                            nc.scalar.copy(dst, tr[:])
                # D2: h1 halves; add the evicted h0 partials and store
                for tt in range(NQT):
                    ot = osb.tile([P, D], F32, tag="o", name="ot")
                    for n2 in range(2):
                        po = psO.tile([P, 512], F32, tag="po", name="po")
                        for sI in range(CI):
                            nc.tensor.matmul(
                                po[:],
                                cf_sb[DH:P, sI, tt * P : (tt + 1) * P],
                                wo_sb[DH:P, sI, n2 * 512 : (n2 + 1) * 512],
                                start=(sI == 0),
                                stop=(sI == CI - 1),
                            )
                        nc.vector.tensor_tensor(
                            ot[:, n2 * 512 : (n2 + 1) * 512],
                            po[:],
                            ev[:, tt, n2, :],
                            mybir.AluOpType.add,
                        )
                    nc.sync.dma_start(out[tt * P : (tt + 1) * P, :], ot[:])
    nc.finalize()
    return nc


def _get_nc():
    if "nc" not in _CACHE:
        _CACHE["nc"] = _build()
    return _CACHE["nc"]


def kernel(x, Wq, Wk, Wv, Wo, bo, **run_kwargs):
    x = np.asarray(x, np.float32)
    Wq = np.asarray(Wq, np.float32)
    Wk = np.asarray(Wk, np.float32)
    Wv = np.asarray(Wv, np.float32)
    Wo = np.asarray(Wo, np.float32)
    bo = np.asarray(bo, np.float32)

    xt16 = np.ascontiguousarray(x.reshape(TQ, D).T).astype(ml_dtypes.bfloat16)
    wo16 = np.ascontiguousarray(
        Wo.reshape(CI, P, D).transpose(1, 0, 2)
    ).astype(ml_dtypes.bfloat16)
    bob = np.ascontiguousarray(np.broadcast_to(bo, (P, D))).astype(np.float32)
    ii = np.arange(P)[:, None]
    jj = np.arange(P)[None, :]
    msk = (jj >= ii).astype(ml_dtypes.bfloat16)
    idn = np.eye(P).astype(ml_dtypes.bfloat16)

    in_maps = []
    for c in range(8):
        sl = slice(P * c, P * (c + 1))
        in_maps.append(
            {
                "xt": xt16,
                "wq": np.ascontiguousarray(
                    Wq[:, sl].reshape(CI, P, P).transpose(1, 0, 2)
                ).astype(ml_dtypes.bfloat16),
                "wk": np.ascontiguousarray(
                    Wk[:, sl].reshape(CI, P, P).transpose(1, 0, 2)
                ).astype(ml_dtypes.bfloat16),
                "wv": np.ascontiguousarray(
                    Wv[:, sl].reshape(CI, P, P).transpose(1, 0, 2)
                ).astype(ml_dtypes.bfloat16),
                "wo": wo16,
                "bob": bob,
                "msk": msk,
                "idn": idn,
            }
        )

    nc = _get_nc()
    res = run_bass_kernel_spmd(nc, in_maps, core_ids=list(range(8)), **run_kwargs)

    outp = np.empty((B, T, D), np.float32)
    for c in range(8):
        b, w = c // 4, c % 4
        outp[b, w * QB : (w + 1) * QB, :] = res.results[c]["out"]
    return outp


# revision 20
# speedup vs baseline: 1.1380x; 1.0075x over previous
"""Multi-head causal attention (b=2, T=2048, d=1024, 16 heads) on 8 TRN2 cores.

Sharding: tensor-parallel over heads, 2 heads per core, both batch elements on
every core.  Per core:
  - QKV projections for its 2 heads (contraction over d_in=1024), with x^T
    resident in SBUF so Q^T/K^T come out in [channel, token] layout; V in
    natural [token, channel] layout augmented with a ones column.
  - Scores in transposed layout S^T[kpos, q]; diagonal tiles shrink N to the
    causally-valid q suffix, and only the leading 128-column chunk of each
    diagonal tile needs the triangular mask.
  - attn @ V computed transposed: ctx[q, d] accumulated per 128-token chunk
    (matmul N = head_dim + 1), so the softmax denominator sits in a PSUM
    column and normalization is a per-partition tensor_scalar multiply.
  - Two 8-core AllToAlls (one per head) re-shard ctx[q, d] windows from
    head-sharded to token-sharded; landed tiles are transposed back to
    [d, q] on the tensor engine (identity transpose) for the out projection.
  - out = ctx @ Wo + bo per 512-token window; the h0-channel half of the
    projection (K=64 accumulation) overlaps AllToAll #2.
Host side only shards/casts inputs and concatenates the 8 output windows.
"""

import sys

sys.path.insert(0, "/opt/trn_rl_repo")

import numpy as np
import ml_dtypes

import concourse.bass as bass
import concourse.mybir as mybir
import concourse.tile as tile
from concourse.tile import add_dep_helper
from concourse import bacc
from concourse.bass_utils import run_bass_kernel_spmd

B = 2
T = 2048
D = 1024
DH = 64
HL = 2  # heads per core
P = 128
CI = D // P  # 8 contraction subtiles
TQ = B * T  # 4096
QB = 512  # q block
NQB = T // QB  # 4 q blocks per batch
NKT = T // P  # 16 kpos tiles per batch
NW = 8  # output windows == cores
NQT = QB // P  # 4 q 128-chunks per window
F32 = mybir.dt.float32
BF16 = mybir.dt.bfloat16
EXP = mybir.ActivationFunctionType.Exp

_CACHE = {}


def _build():
    nc = bacc.Bacc("TRN2", target_bir_lowering=False, num_devices=8)
    xt = nc.dram_tensor("xt", [D, TQ], BF16, kind="ExternalInput")
    wq = nc.dram_tensor("wq", [P, CI, P], BF16, kind="ExternalInput")
    wk = nc.dram_tensor("wk", [P, CI, P], BF16, kind="ExternalInput")
    wv = nc.dram_tensor("wv", [P, CI, P], BF16, kind="ExternalInput")
    wo = nc.dram_tensor("wo", [P, CI, D], BF16, kind="ExternalInput")
    bob = nc.dram_tensor("bob", [P, D], F32, kind="ExternalInput")
    msk = nc.dram_tensor("msk", [P, P], BF16, kind="ExternalInput")
    idn = nc.dram_tensor("idn", [P, P], BF16, kind="ExternalInput")
    out = nc.dram_tensor("out", [QB, D], F32, kind="ExternalOutput")

    xt_r = xt.rearrange("(s p) t -> p s t", p=P)

    with tile.TileContext(nc) as tc:
        with (
            tc.tile_pool(name="const", bufs=1) as const,
            tc.tile_pool(name="dram", bufs=1, space="DRAM") as dram,
        ):
            xt_sb = const.tile([P, CI, TQ], BF16)
            wq_sb = const.tile([P, CI, P], BF16)
            wk_sb = const.tile([P, CI, P], BF16)
            wv_sb = const.tile([P, CI, P], BF16)
            wo_sb = const.tile([P, CI, D], BF16)
            bob_sb = const.tile([P, D], F32)
            msk_sb = const.tile([P, P], BF16)
            idn_sb = const.tile([P, P], BF16)
            q_sb = const.tile([P, TQ], BF16)
            k_sb = const.tile([P, TQ], BF16)
            # V augmented with a trailing ones column (softmax denominator)
            v_sb = const.tile([P, 2 * NKT, HL, DH + 1], BF16)
            # normalized ctx[q, d] per (head, window, q-chunk)
            stg = const.tile([P, HL, NW, NQT, DH], BF16)
            # landed ctx[q, d] tiles for my window: (head, source pair, chunk)
            cfq = const.tile([P, HL, NW, NQT, DH], BF16)
            # transposed ctx^T[d, q] for the out projection; h=0 channels on
            # partitions 0..63, h=1 on 64..127 (global channel 128j+64h+d)
            cf_sb = const.tile([P, CI, QB], BF16)

            # token-chunked x^T DMAs, chained so chunk t8 arrives at ~t8/8 of
            # the transfer; phase A consumes chunks in the same order
            nc.sync.dma_start(wq_sb[:], wq[:])
            nc.sync.dma_start(xt_sb[:, :, 0:P], xt_r[:, :, 0:P])
            nc.sync.dma_start(wk_sb[:], wk[:])
            nc.sync.dma_start(wv_sb[:], wv[:])
            nc.sync.dma_start(msk_sb[:], msk[:])
            xt_spans = [(sub * P, (sub + 1) * P) for sub in range(1, 4)] + [
                (t8 * QB, (t8 + 1) * QB) for t8 in range(1, TQ // QB)
            ]
            for lo, hi in xt_spans:
                nc.sync.dma_start(xt_sb[:, :, lo:hi], xt_r[:, :, lo:hi])
            nc.sync.dma_start(wo_sb[:], wo[:])
            nc.sync.dma_start(bob_sb[:], bob[:])
            nc.sync.dma_start(idn_sb[:], idn[:])
            nc.vector.memset(v_sb[:, :, :, DH : DH + 1], 1.0)

            # ---- Phases 1+2: QKV waves interleaved with attention ----
            # Phase 1: per 512-token wave w, project Q/K/V for that token
            # chunk (all channels), then run h0's attention window w.  This
            # spreads the Act-engine exp load over the QKV span.  Phase 2:
            # h1's attention windows.  One AllToAll per head as before.
            a2a1_in = dram.tile([NW, P, NQT, DH], BF16)
            a2a1_out = dram.tile([NW, P, NQT, DH], BF16)
            a2a2_in = dram.tile([NW, P, NQT, DH], BF16)
            a2a2_out = dram.tile([NW, P, NQT, DH], BF16)
            a2a = ((a2a1_in, a2a1_out), (a2a2_in, a2a2_out))
            land_dma = [None, None]

            # h1 windows whose scores/exp run early, during phase 1 (the
            # Act engine idles there); their at tiles persist in SBUF
            PRE_W = (0, 1, 2, 4, 5, 6)
            PRE_WAVE = {}
            atp = const.tile([P, 24, 2 * QB], BF16)
            pre_at = {}  # (w, group index) -> atp slot

            def win_groups(qb):
                kd = 4 * qb
                gs = [
                    [(k2 * QB, QB, 2 * g + k2) for k2 in range(2)]
                    for g in range(2 * qb)
                ]
                gs.append([(0, 512, kd), (512, 384, kd + 1)])
                gs.append([(0, 256, kd + 2), (256, 128, kd + 3)])
                return gs

            with (
                tc.tile_pool(name="attn", bufs=4) as apool,
                tc.tile_pool(name="psQK", bufs=2, space="PSUM") as psQK,
                tc.tile_pool(name="psS", bufs=2, space="PSUM") as psS,
                tc.tile_pool(name="psCV", bufs=2, space="PSUM") as psCV,
                tc.tile_pool(name="nrm", bufs=2) as nrm,
            ):

                def score_group(h, b, qb, chunks, at=None):
                    hp, tb, qs, kd = DH * h, b * T, b * T + qb * QB, 4 * qb
                    tot = chunks[-1][0] + chunks[-1][1]
                    diag = chunks[0][2] >= kd
                    sps = psS.tile([P, 2 * QB], F32, tag="s", name="sps")
                    if at is None:
                        at = apool.tile([P, 2 * QB], BF16, tag="at", name="at")
                    for off, wN, kt in chunks:
                        qoff = QB - wN  # q columns qoff..512
                        nc.tensor.matmul(
                            sps[:, off : off + wN],
                            k_sb[hp : hp + DH, tb + kt * P : tb + (kt + 1) * P],
                            q_sb[hp : hp + DH, qs + qoff : qs + QB],
                            start=True,
                            stop=True,
                        )
                    nc.scalar.activation(
                        at[:, 0:tot], sps[:, 0:tot], EXP, scale=0.125
                    )
                    if diag:
                        # triangular mask on the leading 128 columns of each
                        # diagonal tile
                        for off, wN, kt in chunks:
                            nc.vector.tensor_mul(
                                at[:, off : off + P],
                                at[:, off : off + P],
                                msk_sb[:],
                            )
                    return at

                def attnv_group(at, h, b, qb, chunks, ctxq):
                    kb, kd = b * NKT, 4 * qb
                    for off, wN, kt in chunks:
                        qoff = QB - wN
                        for qt in range(NQT):
                            if qt * P < qoff:
                                continue  # causally empty
                            nc.tensor.matmul(
                                ctxq[:, qt, :],
                                at[:, off + qt * P - qoff : off + qt * P - qoff + P],
                                v_sb[:, kb + kt, h, :],
                                start=False,
                                stop=(kt == kd + qt),
                                skip_group_check=True,
                            )

                for h in range(HL):
                    hp = DH * h
                    for w in range(NW):
                        b, qb = w // NQB, w % NQB
                        tb = b * T
                        kb = b * NKT
                        qs = tb + qb * QB
                        t8s = slice(w * QB, (w + 1) * QB)
                        kd = 4 * qb  # first diagonal kt
                        # combined ctx + V-projection scratch: one PSUM bank.
                        # ctx chunks at [0:4,0:65]; V scratch at flat [260:388].
                        # start=True would zero the whole 2KB bank, so both
                        # regions are memset once and accumulate start=False.
                        cv = psCV.tile([P, 6, DH + 1], F32, tag="cv", name="cv")
                        cvf = cv[:].rearrange("p a b -> p (a b)")
                        ctxq = cv[:, 0:NQT, :]
                        nc.vector.memset(cvf[:, 0 : NQT * (DH + 1)], 0.0)

                        if h == 0:
                            # QKV projections for this token wave.  Wave 0
                            # runs in 128-column sub-groups so the first
                            # matmuls only wait on a 128-token slice of x^T.
                            for dst, wt, kk in ((q_sb, wq_sb, 0), (k_sb, wk_sb, 1)):
                                pt = psQK.tile([P, QB], F32, tag="qk", name="pt")
                                if w == 0:
                                    nc.vector.memset(pt[:], 0.0)
                                    for sub in range(4):
                                        ss = slice(sub * P, (sub + 1) * P)
                                        for sI in range(CI):
                                            nc.tensor.matmul(
                                                pt[:, ss],
                                                wt[:, sI, :],
                                                xt_sb[:, sI, ss],
                                                start=False,
                                                stop=(sI == CI - 1),
                                                skip_group_check=True,
                                            )
                                else:
                                    for sI in range(CI):
                                        nc.tensor.matmul(
                                            pt[:],
                                            wt[:, sI, :],
                                            xt_sb[:, sI, t8s],
                                            start=(sI == 0),
                                            stop=(sI == CI - 1),
                                        )
                                nc.vector.tensor_copy(dst[:, t8s], pt[:])
                            for tt4 in range(QB // P):
                                tt = w * (QB // P) + tt4
                                pvr = cvf[:, NQT * (DH + 1) : NQT * (DH + 1) + P]
                                nc.vector.memset(pvr, 0.0)
                                for sI in range(CI):
                                    nc.tensor.matmul(
                                        pvr,
                                        xt_sb[:, sI, tt * P : (tt + 1) * P],
                                        wv_sb[:, sI, :],
                                        start=False,
                                        stop=(sI == CI - 1),
                                        skip_group_check=True,
                                    )
                                nc.vector.tensor_copy(
                                    v_sb[:, tt, :, 0:DH],
                                    pvr.rearrange("p (h d) -> p h d", h=HL),
                                )

                        # attention window (h, b, qb)
                        for gi, chunks in enumerate(win_groups(qb)):
                            if h == 1 and (w, gi) in pre_at:
                                at = atp[:, pre_at[(w, gi)], :]
                            else:
                                at = score_group(h, b, qb, chunks)
                            attnv_group(at, h, b, qb, chunks, ctxq)
                        # normalize: per-partition reciprocal of the
                        # denominator column, then scale the d columns
                        rc = nrm.tile([P, NQT, 1], F32, tag="rc", name="rc")
                        nc.vector.reciprocal(rc[:], ctxq[:, :, DH : DH + 1])
                        for qt in range(NQT):
                            nc.vector.tensor_scalar_mul(
                                stg[:, h, w, qt, :],
                                ctxq[:, qt, 0:DH],
                                rc[:, qt, :],
                            )
                        # stage this window for the AllToAll right away
                        nc.sync.dma_start(a2a[h][0][w], stg[:, h, w, :, :])
                        if h == 0:
                            for pw, at_wave in PRE_WAVE.items():
                                if at_wave != w:
                                    continue
                                pb, pqb = pw // NQB, pw % NQB
                                for gi, chunks in enumerate(win_groups(pqb)):
                                    slot = len(pre_at)
                                    pre_at[(pw, gi)] = slot
                                    score_group(1, pb, pqb, chunks, at=atp[:, slot, :])
                    # all 8 windows staged; run this head's AllToAll
                    a_in, a_out = a2a[h]
                    nc.gpsimd.collective_compute(
                        "AllToAll",
                        mybir.AluOpType.bypass,
                        replica_groups=[[0, 1, 2, 3, 4, 5, 6, 7]],
                        ins=[a_in.opt()],
                        outs=[a_out.opt()],
                    )
                    a_out_r = a_out.rearrange("j p a c -> p j a c")
                    nc.sync.dma_start(cfq[:, h, 0:4, :, :], a_out_r[:, 0:4])
                    land_dma[h] = nc.sync.dma_start(
                        cfq[:, h, 4:NW, :, :], a_out_r[:, 4:NW]
                    )
                    if h == 0:
                        # phase 1.5: h1 scores/exp for the precompute windows,
                        # running in A2A#1's shadow
                        for pw in PRE_W:
                            if pw in PRE_WAVE:
                                continue
                            pb, pqb = pw // NQB, pw % NQB
                            for gi, chunks in enumerate(win_groups(pqb)):
                                slot = len(pre_at)
                                pre_at[(pw, gi)] = slot
                                score_group(1, pb, pqb, chunks, at=atp[:, slot, :])

            # ---- Phase D: output projection for my token window ----
            # h0 transposes + D1 (h0 halves, K=64, all 8 units, evicted to
            # SBUF bf16 with the bias pre-added) overlap A2A#2; h1 transposes
            # + D2 (h1 halves) after it lands; final add + store per token
            # tile.  Transposes batch 4 q-chunks per source into one PSUM
            # bank (memset + start=False) so each source needs one copy.
            units = [(tt, n2) for tt in range(NQT) for n2 in range(2)]
            ev = const.tile([P, NQT, 2, 512], BF16)  # h0-half partials + bias
            with (
                tc.tile_pool(name="psO", bufs=6, space="PSUM") as psO,
                tc.tile_pool(name="osb", bufs=2) as osb,
                tc.tile_pool(name="psTT", bufs=2, space="PSUM") as psTT,
            ):
                for hh in range(2):
                    if hh == 1:
                        # D1 between the two transpose rounds (below)
                        for tt, n2 in units:
                            po = psO.tile([P, 512], F32, tag="po", name="po")
                            for sI in range(CI):
                                nc.tensor.matmul(
                                    po[:],
                                    cf_sb[0:DH, sI, tt * P : (tt + 1) * P],
                                    wo_sb[0:DH, sI, n2 * 512 : (n2 + 1) * 512],
                                    start=(sI == 0),
                                    stop=(sI == CI - 1),
                                )
                            nc.vector.tensor_add(
                                ev[:, tt, n2, :],
                                po[:],
                                bob_sb[:, n2 * 512 : (n2 + 1) * 512],
                            )
                    for j in range(NW):
                        for qt in range(NQT):
                            tr = psTT.tile([DH, P], BF16, tag="tr", name="tr")
                            nc.tensor.matmul(
                                tr[:],
                                cfq[:, hh, j, qt, :],
                                idn_sb[:],
                                is_transpose=True,
                            )
                            dst = cf_sb[hh * DH : hh * DH + DH, j, qt * P : (qt + 1) * P]
                            if (j + qt) % 2 == 0:
                                nc.scalar.copy(dst, tr[:])
                            else:
                                nc.vector.tensor_copy(dst, tr[:])
                # D2: h1 halves; add the evicted h0 partials and store
                for tt in range(NQT):
                    ot = osb.tile([P, D], F32, tag="o", name="ot")
                    for n2 in range(2):
                        po = psO.tile([P, 512], F32, tag="po", name="po")
                        for sI in range(CI):
                            nc.tensor.matmul(
                                po[:],
                                cf_sb[DH:P, sI, tt * P : (tt + 1) * P],
                                wo_sb[DH:P, sI, n2 * 512 : (n2 + 1) * 512],
                                start=(sI == 0),
                                stop=(sI == CI - 1),
                            )
                        nc.vector.tensor_tensor(
                            ot[:, n2 * 512 : (n2 + 1) * 512],
                            po[:],
                            ev[:, tt, n2, :],
                            mybir.AluOpType.add,
                        )
                    nc.sync.dma_start(out[tt * P : (tt + 1) * P, :], ot[:])
    nc.finalize()
    return nc


def _get_nc():
    if "nc" not in _CACHE:
        _CACHE["nc"] = _build()
    return _CACHE["nc"]


def kernel(x, Wq, Wk, Wv, Wo, bo, **run_kwargs):
    x = np.asarray(x, np.float32)
    Wq = np.asarray(Wq, np.float32)
    Wk = np.asarray(Wk, np.float32)
    Wv = np.asarray(Wv, np.float32)
    Wo = np.asarray(Wo, np.float32)
    bo = np.asarray(bo, np.float32)

    xt16 = np.ascontiguousarray(x.reshape(TQ, D).T).astype(ml_dtypes.bfloat16)
    wo16 = np.ascontiguousarray(
        Wo.reshape(CI, P, D).transpose(1, 0, 2)
    ).astype(ml_dtypes.bfloat16)
    bob = np.ascontiguousarray(np.broadcast_to(bo, (P, D))).astype(np.float32)
    ii = np.arange(P)[:, None]
    jj = np.arange(P)[None, :]
    msk = (jj >= ii).astype(ml_dtypes.bfloat16)
    idn = np.eye(P).astype(ml_dtypes.bfloat16)

    in_maps = []
    for c in range(8):
        sl = slice(P * c, P * (c + 1))
        in_maps.append(
            {
                "xt": xt16,
                "wq": np.ascontiguousarray(
                    Wq[:, sl].reshape(CI, P, P).transpose(1, 0, 2)
                ).astype(ml_dtypes.bfloat16),
                "wk": np.ascontiguousarray(
                    Wk[:, sl].reshape(CI, P, P).transpose(1, 0, 2)
                ).astype(ml_dtypes.bfloat16),
                "wv": np.ascontiguousarray(
                    Wv[:, sl].reshape(CI, P, P).transpose(1, 0, 2)
                ).astype(ml_dtypes.bfloat16),
                "wo": wo16,
                "bob": bob,
                "msk": msk,
                "idn": idn,
            }
        )

    nc = _get_nc()
    res = run_bass_kernel_spmd(nc, in_maps, core_ids=list(range(8)), **run_kwargs)

    outp = np.empty((B, T, D), np.float32)
    for c in range(8):
        b, w = c // 4, c % 4
        outp[b, w * QB : (w + 1) * QB, :] = res.results[c]["out"]
    return outp
